# revision 11
# baseline (speedup 1.0000x reference)
"""ExpanderGCNNet Bass kernel for 8 Trainium2 NeuronCores.

Strategy: partition nodes (and their incident in-edges, keyed by dst) across
the 8 cores.  Each layer:
  - every core stages its locally-updated node features and an AllGather
    rebuilds the full [N, 128] feature table in each core's HBM,
  - indirect-DMA gathers h[src] rows for the core's (dst-sorted) edge list,
  - per 128-edge chunk a one-hot selection matrix (built on VectorE from an
    iota/is_equal compare) turns segment-sum into TensorE matmuls that
    accumulate in PSUM per 128-dst block (race-free scatter-add),
  - the masked-linear, graph-norm scaling, batchnorm (batch stats via a tiny
    [128,2] AllReduce), relu and residual run on the transposed
    [feat-partition, node-free] layout so all BN ops are per-partition.
Readout folds 1/graph_count into a one-hot graph matmul, AllReduces the
[128,128] partial means and applies the readout linear on every core.
"""

import os
import sys

for _p in ("/opt/trn_rl_repo", os.path.expanduser("~/.axon_site/_ro/trn_rl_repo")):
    if os.path.isdir(_p) and _p not in sys.path:
        sys.path.insert(0, _p)

import numpy as np

import concourse.bacc as bacc
import concourse.bass as bass
import concourse.mybir as mybir
import concourse.tile as tile
from concourse import library_config
from concourse.bass_utils import run_bass_kernel_spmd

F32 = mybir.dt.float32
I32 = mybir.dt.int32
I16 = mybir.dt.int16
AX = mybir.AxisListType.X
ALU = mybir.AluOpType
ACTF = mybir.ActivationFunctionType

P = 128
N_CORES = 8
BN_EPS = 1e-5


MAX_CALL = 8  # chunks per dma_gather call (SWDGE descriptor-ring limit)


def _call_plan(KbL, KbH, cb0):
    """Per-(block, half) gather calls of <= MAX_CALL chunks.

    Returns list of (block, is_high, global_chunk_start, n_chunks)."""
    calls = []
    for b in range(len(KbL)):
        for is_hi, n in ((0, KbL[b]), (1, KbH[b])):
            base = cb0[b] + (KbL[b] if is_hi else 0)
            done = 0
            while done < n:
                k = min(MAX_CALL, n - done)
                calls.append((b, is_hi, int(base + done), int(k)))
                done += k
    return calls


def _default_cfg():
    return dict(
        n_nodes=50000,
        hid=128,
        n_layers=4,
        n_graphs=128,
        piece_chunks=8,
        dbg_no_ag=False,
        dbg_no_bnar=False,
        dbg_no_gather=False,
    )


def _host_prep(inputs, cfg):
    """Shard/reorder inputs on the host; returns (in_maps, meta)."""
    N = cfg["n_nodes"]
    L = cfg["n_layers"]
    LN = (N + N_CORES - 1) // N_CORES
    NB = (LN + P - 1) // P
    NBP = NB * P

    h = np.asarray(inputs["h"], np.float32)
    snorm = np.asarray(inputs["snorm_n"], np.float32).reshape(-1)
    src = np.asarray(inputs["src"], np.int64)
    dst = np.asarray(inputs["dst"], np.int64)
    gids = np.asarray(inputs["graph_ids"], np.int64)

    # ---- edge partition by dst owner; per-core dst-sorted chunk layout ----
    # Edges are keyed (block = dst//128, half = src >= HALF) so int16 gather
    # indices fit; chunk counts per (block, half) are shared across cores.
    NPAD_ = LN * N_CORES
    HALF = NPAD_ // 2
    assert HALF <= 32767
    owner = dst // LN
    per_core = []
    countsL = np.zeros((N_CORES, NB), np.int64)
    countsH = np.zeros((N_CORES, NB), np.int64)
    for c in range(N_CORES):
        m = owner == c
        s = src[m]
        d = dst[m] - c * LN
        b = d >> 7
        hi = (s >= HALF).astype(np.int64)
        key = b * 2 + hi
        o = np.argsort(key, kind="stable")
        s, d, b, hi = s[o], d[o], b[o], hi[o]
        countsL[c] = np.bincount(b[hi == 0], minlength=NB)
        countsH[c] = np.bincount(b[hi == 1], minlength=NB)
        per_core.append((s, d, b, hi))
    KbL = np.maximum(1, -(-countsL.max(axis=0) // P)).astype(np.int64)
    KbH = (-(-countsH.max(axis=0) // P)).astype(np.int64)
    cb0 = np.concatenate([[0], np.cumsum(KbL + KbH)]).astype(np.int64)
    C = int(cb0[-1])

    def wrap16(vals):
        # edge (lane p, chunk s) of a gather call -> Q7 wrap layout
        # W[p % 16, p // 16 + 8 * s], tiled across the 8 Q7 cores.
        k = vals.shape[1]
        A = vals.reshape(8, 16, k)
        B = A.transpose(1, 2, 0)
        W = B.reshape(16, 8 * k)
        return np.tile(W, (8, 1)).astype(np.int16)

    calls = _call_plan(KbL, KbH, cb0)

    src_maps, dst_maps = [], []
    for c in range(N_CORES):
        s, d, b, hi = per_core[c]
        si = np.zeros((P, C), np.int64)
        df = np.full((P, C), -1.0, np.float32)
        sL, dL, bL = s[hi == 0], d[hi == 0], b[hi == 0]
        sH, dH, bH = s[hi == 1] - HALF, d[hi == 1], b[hi == 1]
        for vals_s, vals_d, vals_b, cnts, off in (
            (sL, dL, bL, countsL[c], cb0[:-1]),
            (sH, dH, bH, countsH[c], cb0[:-1] + KbL),
        ):
            starts = np.concatenate([[0], np.cumsum(cnts)])
            r = np.arange(len(vals_d)) - starts[vals_b]
            col = off[vals_b] + (r >> 7)
            row = r & 127
            si[row, col] = vals_s
            df[row, col] = (vals_d & 127).astype(np.float32)
        iw = np.zeros((P, 8 * C), np.int16)
        for (_b, _hi, gc0, k) in calls:
            iw[:, 8 * gc0 : 8 * (gc0 + k)] = wrap16(si[:, gc0 : gc0 + k])
        src_maps.append(iw)
        dst_maps.append(df.astype(np.float32))
    # ---- per-core node data ----
    NPAD = LN * N_CORES
    hp = np.zeros((NPAD, P), np.float32)
    hp[:N] = h
    snp = np.zeros(NPAD, np.float32)
    snp[:N] = snorm
    gp = np.full(NPAD, -1.0, np.float32)
    gp[:N] = gids.astype(np.float32)
    cnt = np.bincount(gids, minlength=cfg["n_graphs"]).astype(np.float32)
    rcnt = 1.0 / np.maximum(cnt, 1.0)
    rcp = np.zeros(NPAD, np.float32)
    rcp[:N] = rcnt[gids]

    # ---- replicated weights / constants ----
    embedWmT = (
        np.asarray(inputs["embed_W"], np.float32)
        * np.asarray(inputs["embed_mask"], np.float32)
    ).T.copy()
    WmT = [
        (
            np.asarray(inputs["layer_W"][l], np.float32)
            * np.asarray(inputs["layer_mask"][l], np.float32)
        ).T.copy()
        for l in range(L)
    ]
    roWT = np.zeros((P, P), np.float32)
    roW = np.asarray(inputs["ro_W"], np.float32)
    n_cls = roW.shape[0]
    roWT[:, :n_cls] = roW.T
    iota = np.tile(np.arange(P, dtype=np.float32), (P, 1))
    ident = np.eye(P, dtype=np.float32)
    w_all = np.stack([embedWmT] + WmT + [roWT, iota, ident]).astype(np.float32)

    nbias = 2 + 3 * L
    bias_all = np.zeros((P, nbias), np.float32)
    bias_all[:, 0] = np.asarray(inputs["embed_b"], np.float32)
    for l in range(L):
        bias_all[:, 1 + l] = np.asarray(inputs["layer_b"][l], np.float32)
        bias_all[:, 1 + L + l] = np.asarray(inputs["gamma"][l], np.float32)
        bias_all[:, 1 + 2 * L + l] = np.asarray(inputs["beta"][l], np.float32)
    bias_all[:n_cls, 1 + 3 * L] = np.asarray(inputs["ro_b"], np.float32)

    in_maps = []
    for c in range(N_CORES):
        lo = c * LN
        snB = np.broadcast_to(snp[lo : lo + LN], (P, LN)).copy()
        snB = np.concatenate([snB, np.zeros((P, NBP - LN), np.float32)], axis=1)
        gid_t = gp[lo : lo + LN]
        gid_t = np.concatenate([gid_t, np.full(NBP - LN, -1.0, np.float32)])
        rc_t = rcp[lo : lo + LN]
        rc_t = np.concatenate([rc_t, np.zeros(NBP - LN, np.float32)])
        in_maps.append(
            {
                "h_local": hp[lo : lo + LN].copy(),
                "src_idx": src_maps[c],
                "dst_f": dst_maps[c],
                "snormB": snB,
                "gid_f": gid_t.reshape(NB, P).T.copy(),
                "rcnt_f": rc_t.reshape(NB, P).T.copy(),
                "w_all": w_all,
                "bias_all": bias_all,
            }
        )

    meta = dict(
        N=N, LN=LN, NB=NB, NBP=NBP, C=C,
        KbL=KbL.tolist(), KbH=KbH.tolist(), cb0=cb0.tolist(),
        HALF=HALF, L=L, n_cls=n_cls, NPAD=NPAD,
    )
    return in_maps, meta


def _build_program(cfg, meta):
    N, LN, NB, NBP, C, L = (
        meta["N"], meta["LN"], meta["NB"], meta["NBP"], meta["C"], meta["L"],
    )
    KbL, KbH = meta["KbL"], meta["KbH"]
    cb0 = meta["cb0"]
    HALF = meta["HALF"]
    NPAD = meta["NPAD"]
    n_cls = meta["n_cls"]
    calls = _call_plan(KbL, KbH, cb0)
    calls_by_block = {}
    for cl in calls:
        calls_by_block.setdefault(cl[0], []).append(cl)
    rg = [list(range(N_CORES))]

    nc = bacc.Bacc("TRN2", target_bir_lowering=False, num_devices=N_CORES)

    h_local = nc.dram_tensor("h_local", [LN, P], F32, kind="ExternalInput")
    src_idx = nc.dram_tensor("src_idx", [P, 8 * C], I16, kind="ExternalInput")
    dst_f = nc.dram_tensor("dst_f", [P, C], F32, kind="ExternalInput")
    snormB_d = nc.dram_tensor("snormB", [P, NBP], F32, kind="ExternalInput")
    gid_d = nc.dram_tensor("gid_f", [P, NB], F32, kind="ExternalInput")
    rcnt_d = nc.dram_tensor("rcnt_f", [P, NB], F32, kind="ExternalInput")
    w_all = nc.dram_tensor("w_all", [L + 4, P, P], F32, kind="ExternalInput")
    bias_all = nc.dram_tensor("bias_all", [P, 2 + 3 * L], F32, kind="ExternalInput")
    y_out = nc.dram_tensor("y_outT", [n_cls, P], F32, kind="ExternalOutput")

    with tile.TileContext(nc) as tc:
        nc.gpsimd.load_library(library_config.mlp)
        with (
            tc.tile_pool(name="state", bufs=1) as st,
            tc.tile_pool(name="wpool", bufs=1) as wp,
            tc.tile_pool(name="epool", bufs=4) as ep,
            tc.tile_pool(name="onehot", bufs=4) as ohp,
            tc.tile_pool(name="work", bufs=3) as wk,
            tc.tile_pool(name="small", bufs=2) as sm,
            tc.tile_pool(name="pag", bufs=2, space="PSUM") as pag,
            tc.tile_pool(name="pz", bufs=2, space="PSUM") as pz,
            tc.tile_pool(name="pt", bufs=2, space="PSUM") as pt,
            tc.tile_pool(name="dram", bufs=1, space="DRAM") as dr,
        ):
            # ---------------- resident loads ----------------
            w_sb = wp.tile([P, (L + 4) * P], F32, tag="w_sb")
            for i in range(L + 4):
                nc.sync.dma_start(out=w_sb[:, i * P : (i + 1) * P], in_=w_all[i])

            def wslice(i):
                return w_sb[:, i * P : (i + 1) * P]

            embedWmT_sb = wslice(0)
            roWT_sb = wslice(L + 1)
            iota_sb = wslice(L + 2)
            ident_sb = wslice(L + 3)

            bias_sb = wp.tile([P, 2 + 3 * L], F32, tag="bias_sb")
            nc.sync.dma_start(out=bias_sb[:], in_=bias_all[:, :])
            eb_sb = bias_sb[:, 0:1]

            def lb_sb(l):
                return bias_sb[:, 1 + l : 2 + l]

            def gam_sb(l):
                return bias_sb[:, 1 + L + l : 2 + L + l]

            def bet_sb(l):
                return bias_sb[:, 1 + 2 * L + l : 2 + 2 * L + l]

            rob_sb = bias_sb[:, 1 + 3 * L : 2 + 3 * L]

            srcI = wp.tile([P, 8 * C], I16, tag="srcI")
            nc.sync.dma_start(out=srcI[:], in_=src_idx[:, :])
            dstF = wp.tile([P, C], F32, tag="dstF")
            nc.sync.dma_start(out=dstF[:], in_=dst_f[:, :])
            snormB = wp.tile([P, NBP], F32, tag="snormB")
            nc.sync.dma_start(out=snormB[:], in_=snormB_d[:, :])
            gidF = wp.tile([P, NB], F32, tag="gidF")
            nc.sync.dma_start(out=gidF[:], in_=gid_d[:, :])
            rcntF = wp.tile([P, NB], F32, tag="rcntF")
            nc.sync.dma_start(out=rcntF[:], in_=rcnt_d[:, :])

            h_T = st.tile([P, NBP], F32, tag="h_T")
            x_sb = st.tile([P, NBP], F32, tag="x_sb")
            s1 = st.tile([P, NB], F32, tag="s1")
            s2 = st.tile([P, NB], F32, tag="s2")
            stats_sb = st.tile([P, 2], F32, tag="stats_sb")
            ar_sb = st.tile([P, 2], F32, tag="ar_sb")
            mu = st.tile([P, 1], F32, tag="mu")
            e2 = st.tile([P, 1], F32, tag="e2")
            msq = st.tile([P, 1], F32, tag="msq")
            var_t = st.tile([P, 1], F32, tag="var_t")
            rstd = st.tile([P, 1], F32, tag="rstd")
            scl = st.tile([P, 1], F32, tag="scl")
            shf = st.tile([P, 1], F32, tag="shf")
            eps_t = st.tile([P, 1], F32, tag="eps_t")
            nc.vector.memset(eps_t[:], BN_EPS)

            def nrows(b):
                return min(P, LN - b * P)

            # ---------------- embed ----------------
            for b in range(NB):
                nb_r = nrows(b)
                ht = wk.tile([P, P], F32, tag="ht")
                if nb_r < P:
                    nc.vector.memset(ht[:], 0.0)
                nc.sync.dma_start(out=ht[:nb_r, :], in_=h_local[b * P : b * P + nb_r, :])
                tp = pt.tile([P, P], F32)
                nc.tensor.transpose(out=tp[:], in_=ht[:], identity=ident_sb)
                tT = wk.tile([P, P], F32, tag="tT")
                nc.vector.tensor_copy(out=tT[:], in_=tp[:])
                zp = pz.tile([P, P], F32, tag="zp")
                nc.tensor.matmul(out=zp[:], lhsT=embedWmT_sb, rhs=tT[:], start=True, stop=True)
                nc.vector.tensor_scalar(
                    out=h_T[:, b * P : (b + 1) * P], in0=zp[:],
                    scalar1=eb_sb, scalar2=None, op0=ALU.add,
                )

            # ---------------- layers ----------------
            for l in range(L):
                # stage local (node-major) shard + AllGather full table
                shard = dr.tile([LN, P], F32, tag=f"shard{l}")
                for b in range(NB):
                    nb_r = nrows(b)
                    tp = pt.tile([P, P], F32)
                    nc.tensor.transpose(
                        out=tp[:], in_=h_T[:, b * P : (b + 1) * P], identity=ident_sb
                    )
                    stg = wk.tile([P, P], F32, tag="stg")
                    nc.vector.tensor_copy(out=stg[:], in_=tp[:])
                    nc.sync.dma_start(
                        out=shard[b * P : b * P + nb_r, :], in_=stg[:nb_r, :]
                    )
                table = dr.tile([NPAD, P], F32, tag=f"table{l}", addr_space="Shared")
                if cfg["dbg_no_ag"]:
                    nc.sync.dma_start(out=table[0:LN, :], in_=shard[:])
                else:
                    nc.gpsimd.collective_compute(
                        "AllGather", ALU.bypass, replica_groups=rg,
                        ins=[shard.opt()], outs=[table.opt()],
                    )

                # per-block gathers + one-hot segment-sum matmuls
                for b in range(NB):
                    agg = pag.tile([P, P], F32)
                    n_chunks = KbL[b] + KbH[b]
                    j = 0
                    if cfg["dbg_no_gather"]:
                        S0 = ohp.tile([P, P], F32, tag="S")
                        nc.vector.tensor_scalar(
                            out=S0[:], in0=iota_sb, scalar1=-2.0,
                            scalar2=None, op0=ALU.is_equal,
                        )
                        nc.tensor.matmul(
                            out=agg[:], lhsT=iota_sb, rhs=S0[:], start=True, stop=True
                        )
                        calls_b = []
                    else:
                        calls_b = calls_by_block[b]
                    for (_b, is_hi, gc0, k) in calls_b:
                        tbl_ap = (
                            table[HALF:NPAD, :] if is_hi else table[0:HALF, :]
                        )
                        et = ep.tile([P, MAX_CALL, P], F32, tag="eblk")
                        nc.gpsimd.dma_gather(
                            et[:, :k, :],
                            tbl_ap,
                            srcI[:, 8 * gc0 : 8 * (gc0 + k)],
                            k * P,
                            k * P,
                            P,
                        )
                        for jj in range(k):
                            c = gc0 + jj
                            S = ohp.tile([P, P], F32, tag="S")
                            nc.vector.tensor_scalar(
                                out=S[:], in0=iota_sb, scalar1=dstF[:, c : c + 1],
                                scalar2=None, op0=ALU.is_equal,
                            )
                            nc.tensor.matmul(
                                out=agg[:], lhsT=et[:, jj, :], rhs=S[:],
                                start=(j == 0), stop=(j == n_chunks - 1),
                            )
                            j += 1
                    aggT = wk.tile([P, P], F32, tag="aggT")
                    nc.vector.tensor_copy(out=aggT[:], in_=agg[:])
                    zp = pz.tile([P, P], F32, tag="zp")
                    nc.tensor.matmul(
                        out=zp[:], lhsT=wslice(1 + l), rhs=aggT[:], start=True, stop=True
                    )
                    # x = (z + b_l) * snorm ; accumulate per-feature sums
                    nc.vector.scalar_tensor_tensor(
                        out=x_sb[:, b * P : (b + 1) * P], in0=zp[:], scalar=lb_sb(l),
                        in1=snormB[:, b * P : (b + 1) * P],
                        op0=ALU.add, op1=ALU.mult,
                        accum_out=s1[:, b : b + 1],
                    )
                    sq = wk.tile([P, P], F32, tag="sq")
                    nc.scalar.activation(
                        out=sq[:], in_=x_sb[:, b * P : (b + 1) * P],
                        func=ACTF.Square, accum_out=s2[:, b : b + 1],
                    )

                # global batchnorm stats
                nc.vector.tensor_reduce(
                    out=stats_sb[:, 0:1], in_=s1[:, :NB], axis=AX, op=ALU.add
                )
                nc.vector.tensor_reduce(
                    out=stats_sb[:, 1:2], in_=s2[:, :NB], axis=AX, op=ALU.add
                )
                stb_in = dr.tile([P, 2], F32, tag=f"stin{l}")
                stb_out = dr.tile([P, 2], F32, tag=f"stout{l}", addr_space="Shared")
                nc.sync.dma_start(out=stb_in[:], in_=stats_sb[:])
                if cfg["dbg_no_bnar"]:
                    nc.sync.dma_start(out=ar_sb[:], in_=stb_in[:])
                else:
                    nc.gpsimd.collective_compute(
                        "AllReduce", ALU.add, replica_groups=rg,
                        ins=[stb_in.opt()], outs=[stb_out.opt()],
                    )
                    nc.sync.dma_start(out=ar_sb[:], in_=stb_out[:])
                inv_n = 1.0 / float(N)
                nc.vector.tensor_scalar(
                    out=mu[:], in0=ar_sb[:, 0:1], scalar1=inv_n, scalar2=None, op0=ALU.mult
                )
                nc.vector.tensor_scalar(
                    out=e2[:], in0=ar_sb[:, 1:2], scalar1=inv_n, scalar2=None, op0=ALU.mult
                )
                nc.vector.tensor_tensor(out=msq[:], in0=mu[:], in1=mu[:], op=ALU.mult)
                nc.vector.tensor_sub(var_t[:], e2[:], msq[:])
                nc.scalar.activation(out=var_t[:], in_=var_t[:], func=ACTF.Sqrt, bias=eps_t[:])
                nc.vector.reciprocal(out=rstd[:], in_=var_t[:])
                nc.vector.tensor_tensor(out=scl[:], in0=gam_sb(l), in1=rstd[:], op=ALU.mult)
                nc.vector.tensor_tensor(out=msq[:], in0=mu[:], in1=scl[:], op=ALU.mult)
                nc.vector.tensor_sub(shf[:], bet_sb(l), msq[:])

                # apply: h += relu(x * scale + shift)
                for b in range(NB):
                    yb = wk.tile([P, P], F32, tag="yb")
                    nc.scalar.activation(
                        out=yb[:], in_=x_sb[:, b * P : (b + 1) * P],
                        func=ACTF.Relu, bias=shf[:], scale=scl[:],
                    )
                    nc.vector.tensor_add(
                        h_T[:, b * P : (b + 1) * P], h_T[:, b * P : (b + 1) * P], yb[:]
                    )

            # ---------------- readout ----------------
            ro_acc = st.tile([P, P], F32, tag="ro_acc")
            nc.vector.memset(ro_acc[:], 0.0)
            for b in range(NB):
                tp = pt.tile([P, P], F32)
                nc.tensor.transpose(
                    out=tp[:], in_=h_T[:, b * P : (b + 1) * P], identity=ident_sb
                )
                hnm = wk.tile([P, P], F32, tag="hnm")
                nc.vector.tensor_copy(out=hnm[:], in_=tp[:])
                G = ohp.tile([P, P], F32, tag="G")
                nc.vector.tensor_scalar(
                    out=G[:], in0=iota_sb, scalar1=gidF[:, b : b + 1],
                    scalar2=rcntF[:, b : b + 1], op0=ALU.is_equal, op1=ALU.mult,
                )
                rp = pz.tile([P, P], F32, tag="zp")
                nc.tensor.matmul(out=rp[:], lhsT=hnm[:], rhs=G[:], start=True, stop=True)
                nc.vector.tensor_add(ro_acc[:], ro_acc[:], rp[:])
            ro_in = dr.tile([P, P], F32, tag="ro_in")
            ro_out = dr.tile([P, P], F32, tag="ro_out", addr_space="Shared")
            nc.sync.dma_start(out=ro_in[:], in_=ro_acc[:])
            nc.gpsimd.collective_compute(
                "AllReduce", ALU.add, replica_groups=rg,
                ins=[ro_in.opt()], outs=[ro_out.opt()],
            )
            hgT = sm.tile([P, P], F32, tag="hgT")
            nc.sync.dma_start(out=hgT[:], in_=ro_out[:])
            op_ = pz.tile([n_cls, P], F32, tag="zp")
            nc.tensor.matmul(
                out=op_[:], lhsT=roWT_sb[:, :n_cls], rhs=hgT[:], start=True, stop=True
            )
            fin = sm.tile([n_cls, P], F32, tag="fin")
            nc.vector.tensor_scalar(
                out=fin[:], in0=op_[:], scalar1=rob_sb[:n_cls, :], scalar2=None, op0=ALU.add
            )
            nc.sync.dma_start(out=y_out[:, :], in_=fin[:])

    nc.compile()
    return nc


_CACHE = {}


def _get_program(cfg, meta):
    key = (tuple(sorted(cfg.items())), meta["C"], tuple(meta["KbL"]), tuple(meta["KbH"]))
    if key not in _CACHE:
        _CACHE[key] = _build_program(cfg, meta)
    return _CACHE[key]


def kernel(_cfg=None, **inputs):
    cfg = _default_cfg()
    if _cfg:
        cfg.update(_cfg)
    cfg["n_nodes"] = int(inputs["h"].shape[0])
    cfg["n_graphs"] = 128
    in_maps, meta = _host_prep(inputs, cfg)
    nc = _get_program(cfg, meta)
    res = run_bass_kernel_spmd(nc, in_maps, core_ids=list(range(N_CORES)))
    outT = res.results[0]["y_outT"]
    return np.ascontiguousarray(outT.T)


if __name__ == "__main__":
    rng = np.random.default_rng(0)
    pass
